# revision 24
# baseline (speedup 1.0000x reference)
"""Trainium2 Bass kernel for chemprop-style MPNN (nn_Cmpd_d_MPNN_3917010174549).

Strategy (8 NeuronCores, data-parallel over bonds/atoms, replicated tables):
  - Directed-bond message table ([n_bonds, 300], fp8e4m3) replicated in each
    core's DRAM; each core computes its 1/8 bond shard, AllGather per step.
  - Atom-factorized message passing: per step, first a_msg[a] = sum_j
    msg[a2b[a,j]] over a 1/8 atom shard (6 gathered rows per atom), AllGather
    the a_msg table (fp8), then per bond u = a_msg[b2a[b]] - msg[b2revb[b]]
    (2 gathered rows per bond) instead of 7 composed gathers per bond.
    This cuts the serialized SWDGE indirect-DMA instruction count (~1us each)
    from 7/bond-tile to 6/atom-tile + 2/bond-tile.
  - Gathers run as [128,1]-offset SWDGE indirect DMAs (one 128-row gather per
    instruction); fp8 tables halve both gather and AllGather HBM traffic.
  - The directed update is fused into PE matmuls over [f_bonds(147);u(300)] @
    [W_i;W_h] with fp32 PSUM; u transposed via PE identity matmuls.
  - Readout: molecule-aligned atom slots; gather 6 rows per slot from the
    final table, fused matmul [f_atoms;1;a_msg] @ [W_o_a;b_o;W_o_h], relu,
    one-hot PE pooling with host-precomputed reciprocal counts.

kernel(**inputs) takes full unsharded inputs, returns [10000, 300] f32.
"""

from dataclasses import dataclass

import numpy as np


# ----------------------------------------------------------------------------
# Configuration
# ----------------------------------------------------------------------------
@dataclass(frozen=True)
class Cfg:
    NC: int = 8            # cores
    NB: int = 300001       # bonds (incl. padding bond 0)
    NA: int = 150001       # atoms (incl. padding atom 0)
    NM: int = 10000        # molecules
    H: int = 300           # hidden
    FB: int = 147          # bond feature dim
    FA: int = 133          # atom feature dim
    MAXNB: int = 6
    DEPTH: int = 3
    CH: int = 8            # bond tiles per chunk
    CHA: int = 8           # atom tiles per chunk
    CHR: int = 8           # readout tiles per chunk
    FP8: bool = True       # table dtype fp8e4m3 (else fp16)

    @property
    def Bs(self):          # bonds per core (multiple of 128)
        return ((self.NB + self.NC - 1) // self.NC + 127) // 128 * 128

    @property
    def NBP(self):
        return self.Bs * self.NC

    @property
    def NT_B(self):        # bond tiles per core
        return self.Bs // 128

    @property
    def As(self):          # atoms per core (multiple of 128)
        return ((self.NA + self.NC - 1) // self.NC + 127) // 128 * 128

    @property
    def NAP(self):
        return self.As * self.NC

    @property
    def NT_A(self):        # atom tiles per core
        return self.As // 128

    @property
    def Mc(self):          # molecules per core
        assert self.NM % self.NC == 0
        return self.NM // self.NC

    @property
    def G_n(self):         # 128-molecule groups per core
        return (self.Mc + 127) // 128


FULL = Cfg()

_BUILD_CACHE: dict = {}


# ----------------------------------------------------------------------------
# Host-side preprocessing
# ----------------------------------------------------------------------------
def prepare_host(inputs: dict, cfg: Cfg):
    """Build per-core input maps. Returns (in_maps, T_g)."""
    f_atoms = np.asarray(inputs["f_atoms"], np.float32)
    f_bonds = np.asarray(inputs["f_bonds"], np.float32)
    a2b = np.asarray(inputs["a2b"], np.int32)
    b2a = np.asarray(inputs["b2a"], np.int32)
    b2revb = np.asarray(inputs["b2revb"], np.int32)
    amol = np.asarray(inputs["atom_mol_id"], np.int32)
    W_i = np.asarray(inputs["W_i"], np.float32)
    W_h = np.asarray(inputs["W_h"], np.float32)
    W_o = np.asarray(inputs["W_o"], np.float32)
    b_o = np.asarray(inputs["b_o"], np.float32)

    NC, NB, NA, NM, H, FB, FA = (
        cfg.NC, cfg.NB, cfg.NA, cfg.NM, cfg.H, cfg.FB, cfg.FA)
    Bs, NT_B, Mc, G_n = cfg.Bs, cfg.NT_B, cfg.Mc, cfg.G_n
    As, NT_A = cfg.As, cfg.NT_A

    # ---- bond side ----
    # padded bond features, transposed per core
    fbT_full = np.zeros((FB, cfg.NBP), np.float16)
    fbT_full[:, :NB] = f_bonds.T.astype(np.float16)

    # per-bond gather rows: rev bond (msg table) and source atom (amsg table)
    rev_full = np.zeros(cfg.NBP, np.int32)
    rev_full[:NB] = b2revb
    ba_full = np.zeros(cfg.NBP, np.int32)
    ba_full[:NB] = b2a                    # atom id == amsg table row

    # ---- atom phase: per-atom 6 incoming-bond rows ----
    at6_full = np.zeros((cfg.NAP, 6), np.int32)
    # atom id a lives at amsg row a; core c computes rows [c*As, (c+1)*As)
    at6_full[:NA] = a2b

    # ---- molecule pooling side ----
    counts = np.bincount(amol, minlength=NM + 1)[:NM]
    atom_ids = np.arange(NA, dtype=np.int64)
    valid = atom_ids != 0
    order = atom_ids[valid][np.argsort(amol[valid], kind="stable")]
    mol_sorted = amol[order]
    starts = np.searchsorted(mol_sorted, np.arange(NM + 1))

    T_g = 1
    for c in range(NC):
        for g in range(G_n):
            m0 = c * Mc + g * 128
            m1 = min(c * Mc + (g + 1) * 128, (c + 1) * Mc)
            T_g = max(T_g, (starts[m1] - starts[m0] + 127) // 128)
    NRT = G_n * T_g

    inv_counts = np.zeros(NM, np.float32)
    nz = counts > 0
    inv_counts[nz] = 1.0 / counts[nz]

    faT_all = np.vstack([f_atoms.T.astype(np.float16),
                         np.ones((1, NA), np.float16)])  # [FA+1, NA]

    wstk = np.vstack([W_i, W_h]).astype(np.float16)
    wr = np.vstack([W_o[:FA], b_o[None, :], W_o[FA:]]).astype(np.float16)

    in_maps = []
    for c in range(NC):
        # bond inputs
        sl = slice(c * Bs, (c + 1) * Bs)
        fbT = np.ascontiguousarray(fbT_full[:, sl])
        fbF = fbT_full  # full table: step 0 is computed redundantly per core
        idxrev = (rev_full[sl].reshape(NT_B, 128).T.copy())
        idxba = (ba_full[sl].reshape(NT_B, 128).T.copy())

        # atom-phase inputs
        sla = slice(c * As, (c + 1) * As)
        idxat = (at6_full[sla]                 # [As, 6]
                 .reshape(NT_A, 128, 6)
                 .transpose(1, 0, 2)
                 .reshape(128, NT_A * 6)
                 .copy())

        # readout inputs
        slot_atom = np.zeros((NRT * 128,), np.int64)
        slot_valid = np.zeros((NRT * 128,), bool)
        oneh = np.zeros((NRT * 128, 128), np.float16)
        for g in range(G_n):
            m0 = c * Mc + g * 128
            m1 = min(c * Mc + (g + 1) * 128, (c + 1) * Mc)
            a0, a1 = starts[m0], starts[m1]
            n = a1 - a0
            base = g * T_g * 128
            slot_atom[base:base + n] = order[a0:a1]
            slot_valid[base:base + n] = True
            oneh[np.arange(base, base + n), mol_sorted[a0:a1] - m0] = 1.0
        idxa = a2b[slot_atom, :6].astype(np.int32)
        idxa[~slot_valid] = 0
        idxa = (idxa.reshape(NRT, 128, 6)
                .transpose(1, 0, 2)
                .reshape(128, NRT * 6)
                .copy())
        faT = faT_all[:, slot_atom].copy()
        faT[:, ~slot_valid] = 0

        invc = np.zeros((128, G_n), np.float32)
        for g in range(G_n):
            m0 = c * Mc + g * 128
            m1 = min(c * Mc + (g + 1) * 128, (c + 1) * Mc)
            invc[: m1 - m0, g] = inv_counts[m0:m1]

        in_maps.append({
            "fbT": fbT, "fbF": fbF,
            "idxrev": idxrev, "idxba": idxba, "idxat": idxat,
            "faT": faT, "idxa": idxa, "oneh": oneh,
            "wstk": wstk, "wr": wr, "invc": invc,
        })
    return in_maps, T_g


# ----------------------------------------------------------------------------
# Device program
# ----------------------------------------------------------------------------
def build_program(cfg: Cfg, T_g: int):
    import concourse.bass as bass
    import concourse.mybir as mybir
    import concourse.tile as tile
    from concourse import bacc
    from concourse.masks import make_identity

    dt = mybir.dt
    DT8 = dt.float8e4 if cfg.FP8 else dt.float16
    NC, H, FB, FA = cfg.NC, cfg.H, cfg.FB, cfg.FA
    Bs, NBP, NT_B, G_n = cfg.Bs, cfg.NBP, cfg.NT_B, cfg.G_n
    As, NAP, NT_A = cfg.As, cfg.NAP, cfg.NT_A
    NRT = G_n * T_g
    CH, CHA, CHR = cfg.CH, cfg.CHA, cfg.CHR
    AluOp = mybir.AluOpType
    ActF = mybir.ActivationFunctionType
    RG = [list(range(NC))]

    h_chunks = []
    o = 0
    while o < H:
        h_chunks.append((o, min(o + 128, H)))
        o += 128
    n_hc = len(h_chunks)

    nc = bacc.Bacc(
        "TRN2", target_bir_lowering=False, debug=False,
        enable_asserts=False, num_devices=NC,
    )

    fbT = nc.dram_tensor("fbT", [FB, Bs], dt.float16, kind="ExternalInput").ap()
    fbF = nc.dram_tensor("fbF", [FB, NBP], dt.float16,
                         kind="ExternalInput").ap()
    idxrev = nc.dram_tensor("idxrev", [128, NT_B], dt.int32,
                            kind="ExternalInput").ap()
    idxba = nc.dram_tensor("idxba", [128, NT_B], dt.int32,
                           kind="ExternalInput").ap()
    idxat = nc.dram_tensor("idxat", [128, NT_A * 6], dt.int32,
                           kind="ExternalInput").ap()
    faT = nc.dram_tensor("faT", [FA + 1, NRT * 128], dt.float16,
                         kind="ExternalInput").ap()
    idxa = nc.dram_tensor("idxa", [128, NRT * 6], dt.int32,
                          kind="ExternalInput").ap()
    oneh = nc.dram_tensor("oneh", [NRT * 128, 128], dt.float16,
                          kind="ExternalInput").ap()
    wstk = nc.dram_tensor("wstk", [FB + H, H], dt.float16,
                          kind="ExternalInput").ap()
    wr = nc.dram_tensor("wr", [FA + 1 + H, H], dt.float16,
                        kind="ExternalInput").ap()
    invc = nc.dram_tensor("invc", [128, G_n], dt.float32,
                          kind="ExternalInput").ap()
    mol_out = nc.dram_tensor("mol_out", [G_n * 128, H], dt.float32,
                             kind="ExternalOutput").ap()

    with tile.TileContext(nc) as tc:
        with (
            tc.tile_pool(name="const", bufs=1) as cpool,
            tc.tile_pool(name="dram", bufs=1, space="DRAM") as dpool,
            tc.tile_pool(name="x", bufs=2) as xpool,
            tc.tile_pool(name="g", bufs=3) as gpool,
            tc.tile_pool(name="u", bufs=2) as upool,
            tc.tile_pool(name="ut", bufs=3) as utpool,
            tc.tile_pool(name="m", bufs=2) as mpool,
            tc.tile_pool(name="pt", bufs=2, space="PSUM") as ptpool,
            tc.tile_pool(name="po", bufs=2, space="PSUM") as popool,
            tc.tile_pool(name="pm", bufs=2, space="PSUM") as pmpool,
        ):
            # ---------------- DRAM internals ----------------
            # msg0 is written by plain per-chunk DMAs (full table computed
            # locally), msg1/msg2 by AllGather (Shared, single writer)
            msgs = [dpool.tile([NBP, H], DT8,
                               addr_space="Local" if s == 0 else "Shared",
                               name=f"msg{s}") for s in range(cfg.DEPTH)]
            shards = [dpool.tile([Bs, H], DT8, name=f"shard{s}")
                      for s in range(cfg.DEPTH)]
            amsgs = [dpool.tile([NAP, H], DT8, addr_space="Shared",
                                name=f"amsg{s}") for s in range(2)]
            ashards = [dpool.tile([As, H], DT8, name=f"ashard{s}")
                       for s in range(2)]

            # ---------------- constants ----------------
            ident = cpool.tile([128, 128], dt.float16, name="ident")
            make_identity(nc, ident[:, :])

            def load_chunks(src_ap, total_rows, name):
                tiles = []
                r = 0
                while r < total_rows:
                    rr = min(r + 128, total_rows)
                    t = cpool.tile([rr - r, H], dt.float16,
                                   name=f"{name}{len(tiles)}")
                    nc.sync.dma_start(out=t[:, :], in_=src_ap[r:rr, :])
                    tiles.append(t)
                    r = rr
                return tiles

            w_x = load_chunks(wstk, FB, "wx")
            w_h = [cpool.tile([b - a, H], dt.float16, name=f"wh{i}")
                   for i, (a, b) in enumerate(h_chunks)]
            for i, (a, b) in enumerate(h_chunks):
                nc.sync.dma_start(out=w_h[i][:, :], in_=wstk[FB + a:FB + b, :])
            wr_x = load_chunks(wr, FA + 1, "wrx")
            wr_h = [cpool.tile([b - a, H], dt.float16, name=f"wrh{i}")
                    for i, (a, b) in enumerate(h_chunks)]
            for i, (a, b) in enumerate(h_chunks):
                nc.sync.dma_start(out=wr_h[i][:, :],
                                  in_=wr[FA + 1 + a:FA + 1 + b, :])

            idxrev_sb = cpool.tile([128, NT_B], dt.int32, name="idxrev_sb")
            nc.sync.dma_start(out=idxrev_sb[:, :], in_=idxrev[:, :])
            idxba_sb = cpool.tile([128, NT_B], dt.int32, name="idxba_sb")
            nc.sync.dma_start(out=idxba_sb[:, :], in_=idxba[:, :])
            idxat_sb = cpool.tile([128, NT_A * 6], dt.int32, name="idxat_sb")
            nc.sync.dma_start(out=idxat_sb[:, :], in_=idxat[:, :])
            idxa_sb = cpool.tile([128, NRT * 6], dt.int32, name="idxa_sb")
            nc.sync.dma_start(out=idxa_sb[:, :], in_=idxa[:, :])
            invc_sb = cpool.tile([128, G_n], dt.float32, name="invc_sb")
            nc.sync.dma_start(out=invc_sb[:, :], in_=invc[:, :])

            # ---------------- helpers ----------------
            def load_x(src, width, t0, ch_t, tagbase):
                tiles = []
                r = 0
                i = 0
                while r < width:
                    rr = min(r + 128, width)
                    xt = xpool.tile([rr - r, ch_t * 128], dt.float16,
                                    tag=f"{tagbase}{i}", name=f"{tagbase}{i}")
                    nc.sync.dma_start(
                        out=xt[:, :],
                        in_=src[r:rr, t0 * 128:(t0 + ch_t) * 128])
                    tiles.append(xt)
                    r = rr
                    i += 1
                return tiles

            def transpose_u(u_slice_fn):
                pt = ptpool.tile([128, n_hc * 128], dt.float16, tag="pt",
                                 name="pt")
                uT = utpool.tile([128, n_hc * 128], dt.float16, tag="uT",
                                 name="uT")
                for i, (a, b) in enumerate(h_chunks):
                    w = b - a
                    nc.tensor.transpose(
                        out=pt[0:w, i * 128:i * 128 + 128],
                        in_=u_slice_fn(a, b),
                        identity=ident[:, :])
                    nc.vector.tensor_copy(
                        out=uT[0:w, i * 128:i * 128 + 128],
                        in_=pt[0:w, i * 128:i * 128 + 128])
                return uT

            def gather6_sum(src, idx_sb, t0, ch_t, out_dt):
                """Gather 6 rows per slot for ch_t tiles from table `src` and
                return the [128, ch_t, H] sum tile (dtype out_dt)."""
                G6 = gpool.tile([128, ch_t * 6, H], DT8, tag="g", name="G6")
                for t in range(ch_t):
                    for j in range(6):
                        nc.gpsimd.indirect_dma_start(
                            out=G6[:, t * 6 + j, :],
                            out_offset=None,
                            in_=src[:, :],
                            in_offset=bass.IndirectOffsetOnAxis(
                                ap=idx_sb[:, (t0 + t) * 6 + j:
                                          (t0 + t) * 6 + j + 1],
                                axis=0),
                        )
                G = G6[:, :, :].rearrange("p (c s) h -> p c s h", s=6)
                S1 = upool.tile([128, ch_t, H], dt.float16, tag="s1", name="S1")
                S2 = upool.tile([128, ch_t, H], dt.float16, tag="s2", name="S2")
                A = upool.tile([128, ch_t, H], out_dt, tag="a6", name="A6")
                tt = nc.vector.tensor_tensor
                tt(out=S1[:, :, :], in0=G[:, :, 0, :], in1=G[:, :, 1, :],
                   op=AluOp.add)
                tt(out=S2[:, :, :], in0=G[:, :, 2, :], in1=G[:, :, 3, :],
                   op=AluOp.add)
                tt(out=S1[:, :, :], in0=S1[:, :, :], in1=S2[:, :, :],
                   op=AluOp.add)
                tt(out=S2[:, :, :], in0=G[:, :, 4, :], in1=G[:, :, 5, :],
                   op=AluOp.add)
                tt(out=A[:, :, :], in0=S1[:, :, :], in1=S2[:, :, :],
                   op=AluOp.add)
                return A

            def bond_matmul_store(step, t0, ch_t, U):
                """inp + U@W_h, relu, store to shard (U None for step 0)."""
                x_tiles = load_x(fbT, FB, t0, ch_t, "x")
                msg_sb = mpool.tile([128, ch_t, H], DT8, tag="msg",
                                    name="msg_sb")
                for t in range(ch_t):
                    po = popool.tile([128, H], dt.float32, tag="po", name="po")
                    n_mm = len(w_x) + (n_hc if U is not None else 0)
                    k = 0
                    for xt, wt in zip(x_tiles, w_x):
                        nc.tensor.matmul(
                            out=po[:, :], lhsT=xt[:, t * 128:(t + 1) * 128],
                            rhs=wt[:, :],
                            start=(k == 0), stop=(k == n_mm - 1))
                        k += 1
                    if U is not None:
                        uT = transpose_u(lambda a, b: U[:, t, a:b])
                        for i, (a, b) in enumerate(h_chunks):
                            w = b - a
                            nc.tensor.matmul(
                                out=po[:, :],
                                lhsT=uT[0:w, i * 128:i * 128 + 128],
                                rhs=w_h[i][:, :],
                                start=(k == 0), stop=(k == n_mm - 1))
                            k += 1
                    nc.scalar.activation(
                        out=msg_sb[:, t, :], in_=po[:, :], func=ActF.Relu)
                dst = shards[step]
                nc.sync.dma_start(
                    out=dst[t0 * 128:(t0 + ch_t) * 128, :].rearrange(
                        "(c p) h -> p c h", p=128),
                    in_=msg_sb[:, :, :])

            # ---------------- step 0: msg0 = relu(f_bonds @ W_i) ----------------
            # computed redundantly for the FULL bond table on every core:
            # ~340us of otherwise-idle PE time buys out the AllGather and its
            # serialization with the step-1 atom gathers
            NT_F = NBP // 128
            for t0 in range(0, NT_F, CH):
                ch_t = min(CH, NT_F - t0)
                x_tiles = load_x(fbF, FB, t0, ch_t, "x")
                msg_sb = mpool.tile([128, ch_t, H], DT8, tag="msg",
                                    name="msg_sb0")
                for t in range(ch_t):
                    po = popool.tile([128, H], dt.float32, tag="po", name="po")
                    k = 0
                    for xt, wt in zip(x_tiles, w_x):
                        nc.tensor.matmul(
                            out=po[:, :], lhsT=xt[:, t * 128:(t + 1) * 128],
                            rhs=wt[:, :],
                            start=(k == 0), stop=(k == len(w_x) - 1))
                        k += 1
                    nc.scalar.activation(
                        out=msg_sb[:, t, :], in_=po[:, :], func=ActF.Relu)
                nc.sync.dma_start(
                    out=msgs[0][t0 * 128:(t0 + ch_t) * 128, :].rearrange(
                        "(c p) h -> p c h", p=128),
                    in_=msg_sb[:, :, :])

            # ---------------- steps 1..DEPTH-1 ----------------
            for step in range(1, cfg.DEPTH):
                src = msgs[step - 1]
                # atom phase: a_msg = sum of 6 incoming bond messages
                for t0 in range(0, NT_A, CHA):
                    ch_t = min(CHA, NT_A - t0)
                    A = gather6_sum(src, idxat_sb, t0, ch_t, DT8)
                    nc.sync.dma_start(
                        out=ashards[step - 1][t0 * 128:(t0 + ch_t) * 128, :]
                        .rearrange("(c p) h -> p c h", p=128),
                        in_=A[:, :, :])
                nc.gpsimd.collective_compute(
                    "AllGather", AluOp.bypass, replica_groups=RG,
                    ins=[ashards[step - 1][:, :]], outs=[amsgs[step - 1][:, :]])

                amsg = amsgs[step - 1]
                for t0 in range(0, NT_B, CH):
                    ch_t = min(CH, NT_B - t0)
                    # rev gathers first: they depend only on msgs[step-1], so
                    # the first chunks overlap the amsg AllGather
                    GR = gpool.tile([128, ch_t, H], DT8, tag="gr", name="GR")
                    for t in range(ch_t):
                        nc.gpsimd.indirect_dma_start(
                            out=GR[:, t, :],
                            out_offset=None,
                            in_=src[:, :],
                            in_offset=bass.IndirectOffsetOnAxis(
                                ap=idxrev_sb[:, t0 + t:t0 + t + 1], axis=0),
                        )
                    GA = gpool.tile([128, ch_t, H], DT8, tag="ga", name="GA")
                    for t in range(ch_t):
                        nc.gpsimd.indirect_dma_start(
                            out=GA[:, t, :],
                            out_offset=None,
                            in_=amsg[:, :],
                            in_offset=bass.IndirectOffsetOnAxis(
                                ap=idxba_sb[:, t0 + t:t0 + t + 1], axis=0),
                        )
                    U = upool.tile([128, ch_t, H], dt.float16, tag="u",
                                   name="U")
                    nc.vector.tensor_tensor(
                        out=U[:, :, :], in0=GA[:, :, :],
                        in1=GR[:, :, :], op=AluOp.subtract)
                    bond_matmul_store(step, t0, ch_t, U)
                nc.gpsimd.collective_compute(
                    "AllGather", AluOp.bypass, replica_groups=RG,
                    ins=[shards[step][:, :]], outs=[msgs[step][:, :]])

            # ---------------- readout ----------------
            src = msgs[cfg.DEPTH - 1]
            for g in range(G_n):
                pm = pmpool.tile([128, H], dt.float32, tag="pm", name="pm")
                for t0 in range(0, T_g, CHR):
                    ch_t = min(CHR, T_g - t0)
                    rt0 = g * T_g + t0
                    U = gather6_sum(src, idxa_sb, rt0, ch_t, dt.float16)
                    xa_tiles = load_x(faT, FA + 1, rt0, ch_t, "xa")
                    for t in range(ch_t):
                        tg = t0 + t
                        ph = popool.tile([128, H], dt.float32, tag="po",
                                         name="ph")
                        n_mm = len(wr_x) + n_hc
                        k = 0
                        for xt, wt in zip(xa_tiles, wr_x):
                            nc.tensor.matmul(
                                out=ph[:, :], lhsT=xt[:, t * 128:(t + 1) * 128],
                                rhs=wt[:, :],
                                start=(k == 0), stop=(k == n_mm - 1))
                            k += 1
                        uT = transpose_u(lambda a, b: U[:, t, a:b])
                        for i, (a, b) in enumerate(h_chunks):
                            w = b - a
                            nc.tensor.matmul(
                                out=ph[:, :],
                                lhsT=uT[0:w, i * 128:i * 128 + 128],
                                rhs=wr_h[i][:, :],
                                start=(k == 0), stop=(k == n_mm - 1))
                            k += 1
                        ah = mpool.tile([128, H], dt.float16, tag="ah",
                                        name="ah")
                        nc.scalar.activation(out=ah[:, :], in_=ph[:, :],
                                             func=ActF.Relu)
                        oh = mpool.tile([128, 128], dt.float16, tag="oh",
                                        name="oh")
                        rt = g * T_g + tg
                        nc.sync.dma_start(
                            out=oh[:, :],
                            in_=oneh[rt * 128:(rt + 1) * 128, :])
                        nc.tensor.matmul(
                            out=pm[:, :], lhsT=oh[:, :], rhs=ah[:, :],
                            start=(tg == 0), stop=(tg == T_g - 1))
                out_sb = mpool.tile([128, H], dt.float32, tag="osb",
                                    name="out_sb")
                nc.vector.tensor_scalar_mul(
                    out=out_sb[:, :], in0=pm[:, :],
                    scalar1=invc_sb[:, g:g + 1])
                nc.sync.dma_start(out=mol_out[g * 128:(g + 1) * 128, :],
                                  in_=out_sb[:, :])

    nc.compile()
    return nc


# ----------------------------------------------------------------------------
# Entry point
# ----------------------------------------------------------------------------
def _get_program(cfg: Cfg, T_g: int):
    key = (cfg, T_g)
    if key not in _BUILD_CACHE:
        _BUILD_CACHE[key] = build_program(cfg, T_g)
    return _BUILD_CACHE[key]


def run_on_hw(nc, in_maps, cfg, trace=False):
    from concourse.bass_utils import run_bass_kernel_spmd
    res = run_bass_kernel_spmd(nc, in_maps, list(range(cfg.NC)), trace=trace)
    return res


def assemble_output(results, cfg: Cfg):
    out = np.zeros((cfg.NM, cfg.H), np.float32)
    for c in range(cfg.NC):
        out[c * cfg.Mc:(c + 1) * cfg.Mc] = results[c]["mol_out"][:cfg.Mc]
    return out


def kernel(**inputs) -> np.ndarray:
    cfg = FULL
    in_maps, T_g = prepare_host(inputs, cfg)
    nc = _get_program(cfg, T_g)
    res = run_on_hw(nc, in_maps, cfg)
    return assemble_output(res.results, cfg)
